# revision 65
# baseline (speedup 1.0000x reference)
"""GQA causal attention (B=2,S=2048,HID=2048,H=16,KVH=4,D=128) on 8 trn2 cores.

Sharding: core = b*4 + g  (b: batch, g: head-group of 4 Q heads + 1 KV head).
Per-core kernel computes q/k/v projections (+RoPE), causal softmax attention
for its 4 heads, and a partial output projection; host sums the 4 partials
per batch.

Layout strategy (all matmuls bf16 with fp32 PSUM accumulation):
  - hiddenT [HID, S] per batch; weights pre-transposed on host.
  - Projections (N=1024 streams) produce qT/kT/vT [dims, S]; RoPE applied in
    qT layout (rotate-half = partition-shifted DVE copy).
  - scoresT[sj, si] = kT_slice.T @ qT -> ACT exp (1/sqrt(D) folded into the
    activation scale; no max-subtraction: scores are O(+-10) so fp32 exp is
    safe) -> bf16 expT tiles.
  - causal mask folded into the score matmul: diagonal blocks get a second
    accumulating matmul (-1024*I) @ tri_upper so masked logits exp to 0 —
    keeps the DVE out of the score->exp->den/av chain.
  - attention inner loop software-pipelined 4 deep as one continuous
    (head, jb) stream (scores+exp for pair i+4 issued before den/av of
    pair i) so the PE never stalls on ACT's exp.
  - AV with v stationary: avT[d, si] += v[sj,:].T @ expT[sj, si] — result
    lands directly in the xT layout needed by the output projection.
  - softmax denominators: PE ones-vector matmuls (M=1) accumulate
    den[1, si] in PSUM alongside AV, then DVE reciprocal + gpsimd
    partition_broadcast (no DRAM bounce) + DVE mul.
  - RoPE: rotate-half fused into the sin multiplies via cross-partition
    operand reads (3 DVE ops per stream instead of 5).
  - chunk-level schedule keeps the PE dense: per iteration c emit
    proj(c) -> attn(c-1) -> outproj(c-2), so every chunk's norm chain
    drains under following work and the PE p-state never drops. h(c+1)
    prefetched during iteration c; wq/h(0) streamed in 8 pieces so the
    first projection starts ~2 us in.
  - outproj PSUM drains alternate ACT/DVE per j (gpsimd cannot read PSUM);
    the very last s-block streams its store per-j to shorten the tail.
  - DMA queues: weights + v transposes on SP HWDGE, h + out on ACT HWDGE,
    bf16 cos/sin (chunk-0 slice first) on the Pool SWDGE.
"""

import math
import numpy as np
import ml_dtypes

B, S, HID = 2, 2048, 2048
H, KVH, D = 16, 4, 128
GROUPS = 4            # head groups == KV heads
HD_PER_G = 4          # query heads per group
N_CORES = 8
P = 128
HIDC = HID // P       # 16 hid chunks
W = 512               # matmul chunk (psum bank limit: 512 f32)
WC = S // W           # 4 s-chunks
SB = S // P           # 16 s-blocks of 128

BF16 = ml_dtypes.bfloat16
_CACHE = {}


def build_nc(repeat=1, loop_n=None, internal_inputs=False, norm_mode="gpsimd"):
    """loop_n: if set, wrap the body in a hardware For_i loop (for timing).
    internal_inputs: declare inputs as internal DRAM (garbage data, no host
    upload) — timing-only variant.
    norm_mode: 'dma' (recip + DMA partition-broadcast + mul), 'gpsimd'
    (recip+gpsimd partition_broadcast+mul) or 'none' (timing-only:
    xT = copy(avT), skips normalization)."""
    import contextlib
    import concourse.bass as bass
    import concourse.tile as tile
    from concourse import bacc, mybir
    from concourse.bass_isa import ReduceOp

    f32 = mybir.dt.float32
    bf16 = mybir.dt.bfloat16

    nc = bacc.Bacc("TRN2", target_bir_lowering=False, debug=False,
                   num_devices=N_CORES)

    def din(name, shape, dt):
        if internal_inputs:
            return nc.dram_tensor(name, shape, dt).ap()
        return nc.dram_tensor(name, shape, dt, kind="ExternalInput").ap()
    rd_scr = nc.dram_tensor("rd_scr", [WC, HD_PER_G, W], f32).ap()
    hT = din("hT", [HID, S], bf16)
    wqT = din("wqT", [HID, HD_PER_G * D], bf16)
    wkT = din("wkT", [HID, D], bf16)
    wvT = din("wvT", [HID, D], bf16)
    woT = din("woT", [HD_PER_G * D, HID], bf16)
    cosT = din("cosT", [D, S], bf16)
    sinT = din("sinT", [D, S], bf16)
    maskT = din("maskT", [P, P], bf16)
    ident = din("ident", [P, P], bf16)
    # out stays a real ExternalOutput in the timing variant too, so the
    # compiler cannot dead-code-eliminate the body
    out = nc.dram_tensor("out", [S, HID], bf16, kind="ExternalOutput").ap()
    sink = None

    inv_sqrt_d = 1.0 / math.sqrt(D)

    with tile.TileContext(nc) as tc:
        with (
            tc.tile_pool(name="consts", bufs=1) as consts,
            tc.tile_pool(name="persist", bufs=1) as persist,
            tc.tile_pool(name="hpool", bufs=3) as hpool,
            tc.tile_pool(name="rope", bufs=2) as rope,
            tc.tile_pool(name="expp", bufs=8) as expp,
            tc.tile_pool(name="rcp", bufs=3) as rcp,
            tc.tile_pool(name="outp", bufs=2) as outp,
            tc.tile_pool(name="ps512", bufs=4, space="PSUM") as ps512,
            tc.tile_pool(name="avt", bufs=2, space="PSUM") as avtp,
            tc.tile_pool(name="denp", bufs=2, space="PSUM") as denp,
        ):
            # ---- constant loads --------------------------------------
            # SP queue: weights in first-use order (wq halves -> wk -> wv ->
            # tri/negid -> woT); h + out on the ACT HWDGE queue; cos/sin in
            # per-chunk slices on the PE queue; norm bounce on the DVE queue.
            wqT_sb = consts.tile([P, HIDC, HD_PER_G * D], bf16)
            wq_re = wqT.rearrange("(c p) d -> p c d", p=P)
            for qq in range(8):
                qsl = slice(qq * HIDC // 8, (qq + 1) * HIDC // 8)
                nc.sync.dma_start(out=wqT_sb[:, qsl], in_=wq_re[:, qsl])
            wkT_sb = consts.tile([P, HIDC, D], bf16)
            nc.sync.dma_start(out=wkT_sb, in_=wkT.rearrange("(c p) d -> p c d", p=P))
            wvT_sb = consts.tile([P, HIDC, D], bf16)
            nc.sync.dma_start(out=wvT_sb, in_=wvT.rearrange("(c p) d -> p c d", p=P))
            cosT_sb = consts.tile([P, S], bf16)
            sinT_sb = consts.tile([P, S], bf16)
            # chunk-0 slice first so the first rope isn't gated on the
            # full 2 MB table transfer
            nc.gpsimd.dma_start(out=cosT_sb[:, 0:W], in_=cosT[:, 0:W])
            nc.gpsimd.dma_start(out=sinT_sb[:, 0:W], in_=sinT[:, 0:W])
            nc.gpsimd.dma_start(out=cosT_sb[:, W:], in_=cosT[:, W:])
            nc.gpsimd.dma_start(out=sinT_sb[:, W:], in_=sinT[:, W:])
            tri_sb = consts.tile([P, P], bf16)
            nc.sync.dma_start(out=tri_sb, in_=maskT)
            negid_sb = consts.tile([P, P], bf16)
            nc.sync.dma_start(out=negid_sb, in_=ident)
            ones_sb = consts.tile([P, 1], bf16)
            nc.vector.memset(ones_sb, 1.0)
            woT_sb = consts.tile([P, HD_PER_G, HID], bf16)
            nc.sync.dma_start(out=woT_sb, in_=woT.rearrange("(m p) h -> p m h", p=P))

            # ---- persistent intermediates ----------------------------
            qrT_sb = persist.tile([P, HD_PER_G, S], bf16)   # rotated qT per head
            krT_sb = persist.tile([P, S], bf16)             # rotated kT
            vT_sb = persist.tile([P, S], bf16)              # vT (pre-transpose)
            v_nat = persist.tile([P, SB, D], bf16)          # v natural [sj, d]
            xT_sb = persist.tile([P, HD_PER_G, S], bf16)    # attn out (transposed)

            def rope_chunk(ps, dst_ap, c):
                """dst = ps*cos + rot_half(ps)*sin_signed on wide chunk c."""
                sl = slice(c * W, (c + 1) * W)
                t1 = rope.tile([P, W], f32, tag="t1")
                nc.vector.tensor_mul(t1, ps, cosT_sb[:, sl])
                t2 = rope.tile([P, W], f32, tag="t2")
                nc.vector.tensor_mul(t2[0:64, :], ps[64:128, :],
                                     sinT_sb[0:64, sl])
                nc.vector.tensor_mul(t2[64:128, :], ps[0:64, :],
                                     sinT_sb[64:128, sl])
                nc.vector.tensor_add(dst_ap, t1, t2)

            if internal_inputs:
                # timing-only: fill internal inputs with finite values
                zb = consts.tile([P, S], bf16, tag="zb")
                nc.vector.memset(zb, 0.01)
                zf1 = consts.tile([P, S], f32, tag="zf1")
                nc.vector.memset(zf1, 1.0)
                zf0 = consts.tile([P, S], f32, tag="zf0")
                nc.vector.memset(zf0, 0.0)
                for cc in range(HIDC):
                    hrc = hT.rearrange("(c p) s -> c p s", p=P)
                    nc.sync.dma_start(out=hrc[cc], in_=zb)
                    nc.sync.dma_start(
                        out=wqT.rearrange("(c p) d -> c p d", p=P)[cc],
                        in_=zb[:, 0:HD_PER_G * D])
                    nc.sync.dma_start(
                        out=wkT.rearrange("(c p) d -> c p d", p=P)[cc],
                        in_=zb[:, 0:D])
                    nc.sync.dma_start(
                        out=wvT.rearrange("(c p) d -> c p d", p=P)[cc],
                        in_=zb[:, 0:D])
                for cc in range(HD_PER_G):
                    nc.sync.dma_start(
                        out=woT.rearrange("(m p) h -> m p h", p=P)[cc], in_=zb)
                nc.sync.dma_start(out=cosT, in_=zb)
                nc.sync.dma_start(out=sinT, in_=zb)
                nc.sync.dma_start(out=maskT, in_=zb[:, 0:P])
                nc.sync.dma_start(out=ident, in_=zb[:, 0:P])

            loop_cm = (tc.For_i(0, loop_n, 1) if loop_n is not None
                       else contextlib.nullcontext())

            hre = hT.rearrange("(k p) s -> p k s", p=P)

            def load_h(c, pieces=2):
                ssl = slice(c * W, (c + 1) * W)
                h_sb = hpool.tile([P, HIDC, W], bf16, tag="h")
                for qq in range(pieces):
                    ksl = slice(qq * HIDC // pieces, (qq + 1) * HIDC // pieces)
                    nc.scalar.dma_start(out=h_sb[:, ksl], in_=hre[:, ksl, ssl])
                return h_sb

            def proj_chunk(c, h_sb):
                """q/k/v projections + rope for s-chunk c, v transposes."""
                ssl = slice(c * W, (c + 1) * W)

                def proj(w_sb, dsl):
                    ps = ps512.tile([P, W], f32, tag="ps512")
                    for k in range(HIDC):
                        nc.tensor.matmul(
                            ps, w_sb[:, k, dsl], h_sb[:, k, :],
                            start=(k == 0), stop=(k == HIDC - 1))
                    return ps

                for hd in range(HD_PER_G):   # Q
                    ps = proj(wqT_sb, slice(hd * D, (hd + 1) * D))
                    rope_chunk(ps, qrT_sb[:, hd, ssl], c)
                ps = proj(wkT_sb, slice(0, D))          # K
                rope_chunk(ps, krT_sb[:, ssl], c)
                ps = proj(wvT_sb, slice(0, D))          # V
                nc.vector.tensor_copy(vT_sb[:, ssl], ps)

                # v transposes for this chunk's new sj blocks (DMA xbar, SP
                # queue: consumed by av matmuls one iteration later)
                for jb in range(c * (W // P), (c + 1) * (W // P)):
                    nc.sync.dma_start_transpose(
                        out=v_nat[:, jb, :],
                        in_=vT_sb[:, jb * P:(jb + 1) * P])

            def attn_chunk(c):
                """softmax attention for si-chunk c (k/v chunks <= c ready).

                Software-pipelined: scores+exp for jb run LA=2 blocks ahead
                of den/av so the PE never waits on ACT's exp."""
                base = c * W
                jbmax = (c + 1) * (W // P)
                LA = 4
                avts, dens, ets = {}, {}, {}

                def emit_score(hd, jb):
                    si_start = max(base, jb * P)
                    off = si_start - base
                    wd = W - off
                    diag = jb * P >= base
                    pss = ps512.tile([P, W], f32, tag="ps512")
                    nc.tensor.matmul(
                        pss[:, :wd],
                        krT_sb[:, jb * P:(jb + 1) * P],
                        qrT_sb[:, hd, si_start:base + W],
                        start=True, stop=not diag,
                        skip_group_check=True)
                    if diag:
                        # causal mask: += (-1024*I) @ tri_upper on the
                        # diagonal 128x128 -> exp underflows to 0
                        nc.tensor.matmul(
                            pss[:, 0:P], negid_sb, tri_sb,
                            start=False, stop=True,
                            skip_group_check=True)
                    et = expp.tile([P, W], bf16, tag="expT")
                    nc.scalar.activation(
                        et[:, :wd], pss[:, :wd],
                        func=mybir.ActivationFunctionType.Exp,
                        scale=inv_sqrt_d)
                    ets[hd, jb] = (et, off, wd)

                def emit_denav(hd, jb):
                    et, off, wd = ets.pop((hd, jb))
                    if jb == 0:
                        avts[hd] = avtp.tile([P, W], f32, tag="avt",
                                             name=f"avt{hd}")
                        dens[hd] = denp.tile([1, W], f32, tag="den1",
                                             name=f"den{hd}")
                    nc.tensor.matmul(
                        dens[hd][:, off:], ones_sb, et[:, :wd],
                        start=(jb == 0), stop=(jb == jbmax - 1),
                        skip_group_check=True)
                    nc.tensor.matmul(
                        avts[hd][:, off:], v_nat[:, jb, :], et[:, :wd],
                        start=(jb == 0), stop=(jb == jbmax - 1),
                        skip_group_check=True)
                    if jb == jbmax - 1:
                        emit_norm(hd)

                def emit_norm(hd):
                    avT, den1 = avts.pop(hd), dens.pop(hd)
                    if norm_mode == "none":
                        nc.vector.tensor_copy(
                            xT_sb[:, hd, base:base + W], avT)
                        return
                    rden_f = rcp.tile([1, W], f32, tag="rdenf")
                    nc.vector.reciprocal(rden_f, den1)
                    rbc_s = rcp.tile([P, W], f32, tag="rbcs")
                    if norm_mode == "dma":
                        nc.sync.dma_start(
                            out=rd_scr[c, hd], in_=rden_f)
                        nc.sync.dma_start(
                            out=rbc_s,
                            in_=rd_scr[c, hd].partition_broadcast(P))
                    else:
                        nc.gpsimd.partition_broadcast(rbc_s, rden_f, P)
                    nc.vector.tensor_mul(
                        xT_sb[:, hd, base:base + W], avT, rbc_s)

                # one continuous (hd, jb) pipeline across head boundaries
                pairs = [(hd, jb) for hd in range(HD_PER_G)
                         for jb in range(jbmax)]
                for i, (hd, jb) in enumerate(pairs):
                    emit_score(hd, jb)
                    if i >= LA:
                        emit_denav(*pairs[i - LA])
                for hd, jb in pairs[len(pairs) - LA:]:
                    emit_denav(hd, jb)

            def outproj(c, last=False):
                """output projection for chunk c's s-blocks. PSUM drains
                alternate ACT/DVE so banks recycle fast; the very last
                s-block streams its store per-j to shorten the tail."""
                for sb in range(c * (W // P), (c + 1) * (W // P)):
                    out_t = outp.tile([P, HID], bf16, tag="out")
                    fin = last and sb == (c + 1) * (W // P) - 1
                    for j in range(HID // W):
                        pso = ps512.tile([P, W], f32, tag="ps512")
                        for m in range(HD_PER_G):
                            nc.tensor.matmul(
                                pso, xT_sb[:, m, sb * P:(sb + 1) * P],
                                woT_sb[:, m, j * W:(j + 1) * W],
                                start=(m == 0), stop=(m == HD_PER_G - 1))
                        jo = j * W
                        if j % 2 == 0:
                            nc.scalar.copy(out_t[:, jo:jo + W], pso)
                        else:
                            nc.vector.tensor_copy(out_t[:, jo:jo + W], pso)
                        if fin:
                            # SP queue: ACT SEQ is busy with the drains and
                            # each DMA issue costs ~1.2 us of sequencer time
                            nc.sync.dma_start(
                                out=out[sb * P:(sb + 1) * P, jo:jo + W],
                                in_=out_t[:, jo:jo + W])
                    if not fin:
                        nc.scalar.dma_start(
                            out=out[sb * P:(sb + 1) * P, :], in_=out_t)

            with loop_cm:
              for _rep in range(repeat):
                h_tiles = {}
                for c in range(WC):
                    if c == 0:
                        h_tiles[0] = load_h(0, pieces=8)
                    if c + 1 < WC:
                        h_tiles[c + 1] = load_h(c + 1)
                    proj_chunk(c, h_tiles.pop(c))
                    if c >= 1:
                        attn_chunk(c - 1)
                    if c >= 2:
                        outproj(c - 2)
                attn_chunk(WC - 1)
                outproj(WC - 2)
                outproj(WC - 1, last=True)




    nc.compile()
    return nc


def _prep_inputs(hidden_states, cos, sin, wq, wk, wv, wo):
    """Host-side shard + layout prep. Returns in_maps for cores 0..7."""
    hidden_states = np.asarray(hidden_states, dtype=np.float32)
    cos = np.asarray(cos, dtype=np.float32)
    sin = np.asarray(sin, dtype=np.float32)
    wq = np.asarray(wq, dtype=np.float32)
    wk = np.asarray(wk, dtype=np.float32)
    wv = np.asarray(wv, dtype=np.float32)
    wo = np.asarray(wo, dtype=np.float32)

    cosT = np.ascontiguousarray(cos[:, 0, :].T).astype(BF16)   # [D, S]
    sinT_full = sin[:, 0, :].T                                  # [D, S]
    sinT = np.concatenate([-sinT_full[:64], sinT_full[64:]], axis=0)
    sinT = np.ascontiguousarray(sinT).astype(BF16)

    # strict upper triangle (sj > si): rows = partition (sj), cols = si
    mask = (np.arange(P)[:, None] > np.arange(P)[None, :]).astype(BF16)
    identity = (-1024.0 * np.eye(P)).astype(BF16)

    hTs = [np.ascontiguousarray(hidden_states[b].T).astype(BF16)
           for b in range(B)]

    in_maps = []
    for core in range(N_CORES):
        b, g = divmod(core, GROUPS)
        qsl = slice(g * HD_PER_G * D, (g + 1) * HD_PER_G * D)
        ksl = slice(g * D, (g + 1) * D)
        in_maps.append({
            "hT": hTs[b],
            "wqT": np.ascontiguousarray(wq[qsl, :].T).astype(BF16),
            "wkT": np.ascontiguousarray(wk[ksl, :].T).astype(BF16),
            "wvT": np.ascontiguousarray(wv[ksl, :].T).astype(BF16),
            "woT": np.ascontiguousarray(wo[:, qsl].T).astype(BF16),
            "cosT": cosT,
            "sinT": sinT,
            "maskT": mask,
            "ident": identity,
        })
    return in_maps


def kernel(hidden_states, cos, sin, wq, wk, wv, wo):
    from concourse.bass_utils import run_bass_kernel_spmd

    if "nc" not in _CACHE:
        _CACHE["nc"] = build_nc()
    nc = _CACHE["nc"]

    in_maps = _prep_inputs(hidden_states, cos, sin, wq, wk, wv, wo)
    res = run_bass_kernel_spmd(nc, in_maps, core_ids=list(range(N_CORES)))

    out = np.zeros((B, S, HID), dtype=np.float32)
    for core in range(N_CORES):
        b = core // GROUPS
        out[b] += res.results[core]["out"].astype(np.float32)
    return out
